# revision 29
# baseline (speedup 1.0000x reference)
"""Trainium2 Bass kernel for nn_LSM_IniReconNet.

The reference computes, per contiguous 16-element block of the signal,
z = W1 @ block then y = W2 @ z — i.e. a fixed 16x16 linear map
M = W2 @ W1 applied blockwise. This is pure streaming (memory-bound):
every element is read once, transformed by M, written once.

v2 strategy:
  * bf16 on the wire both directions (rel-err gate is 2e-2; bf16
    end-to-end lands ~4e-3), halving HBM traffic per core to
    4 MB in + 4 MB out.
  * The host lays each core's slice out as [128 partitions = signal
    position within a 128-superblock, free = (superblock, row)] so the
    contraction dim is already on partitions: the device needs NO
    transposes — just DMA in, one bf16 matmul per [128,512] chunk
    against the constant K = kron(I8, M.T), a PSUM->SBUF copy (casting
    back to bf16), and DMA out. The host inverts the permutation.
  * HWDGE DMAs: loads on nc.sync (SP ring), stores on nc.scalar (ACT
    ring) — separate rings, concurrent in/out streaming, ~0.6us fixed
    cost vs ~2us for the SWDGE path.

Sharding: pure data parallel — batch rows split across 8 cores, K
replicated.
"""

import sys

for _p in ("/opt/trn_rl_repo", "/root/.axon_site/_ro/trn_rl_repo"):
    if _p not in sys.path:
        sys.path.insert(0, _p)

import ml_dtypes
import numpy as np

import concourse.bass as bass
import concourse.mybir as mybir
from concourse.bass_utils import run_bass_kernel_spmd
from concourse.tile import TileContext

F32 = mybir.dt.float32
BF16 = mybir.dt.bfloat16
NPBF16 = np.dtype(ml_dtypes.bfloat16)

NB = 4096  # batch
H = 4096  # signal length
BLOCK = 16
SP = 8
N_CORES = 8
ROWS_PER_CORE = NB // N_CORES  # 512
NSUPER = H // 128  # 32 superblocks of 128 positions per row
NGROUPS = 4  # DMA granularity: 1 MB bf16 per group
CHUNKS_PER_GROUP = (NSUPER * ROWS_PER_CORE // 512) // NGROUPS  # 8
FREE = NSUPER * ROWS_PER_CORE  # 16384 free columns on chip

_NC_CACHE = {}


def _split_multi_waits(nc):
    """walrus codegen accepts at most one semaphore wait per instruction
    (beyond what same-queue elision removes). Tile attaches several — most
    notably on the kernel-tail drain. Hoist all but one wait onto wait-only
    NOPs placed immediately before the instruction on the same engine queue.
    """
    ctr = 0
    for fn in nc.m.functions:
        for blk in fn.blocks:
            old = list(blk.instructions)
            if not any(
                i.sync_info is not None and len(i.sync_info.on_wait) > 1 for i in old
            ):
                continue
            new = []
            for inst in old:
                si = inst.sync_info
                if si is not None and len(si.on_wait) > 1:
                    waits = list(si.on_wait)
                    for w in waits[:-1]:
                        ctr += 1
                        new.append(
                            mybir.InstNoOp(
                                name=f"I-waitsplit-{ctr}",
                                sync_info=mybir.SyncInfo(on_wait=[w], on_update=[]),
                                bass_nofuse=True,
                                engine=inst.engine,
                            )
                        )
                    inst.sync_info = mybir.SyncInfo(
                        on_wait=[waits[-1]], on_update=list(si.on_update)
                    )
                new.append(inst)
            blk.instructions = new
    return nc


def _build():
    """Per-core SPMD program.

    x: (128, FREE) bf16 — partition p holds position (128*c + p) of the
    signal for superblock c, free col c*512+n is batch row n.
    k: (128, 128) bf16 = kron(I8, M.T).  y: same layout as x.
    """
    nc = bass.Bass()
    gcols = FREE // NGROUPS  # 4096 free cols per DMA group
    nhalf = FREE // 2048  # 8 output blocks of 512 KB
    # DRAM layouts are block-contiguous so every DMA is a plain
    # contiguous-DRAM <-> [128, N]-SBUF transfer (the cheap 9-desc/engine
    # swizzle; a [128, slice] view of a row-major array would need
    # per-partition descriptors instead).
    x = nc.declare_dram_parameter("x", [NGROUPS, 128, gcols], BF16, isOutput=False)
    k = nc.declare_dram_parameter("k", [128, 128], BF16, isOutput=False)
    y = nc.declare_dram_parameter("y", [NGROUPS, 128, gcols], BF16, isOutput=True)

    with TileContext(nc) as tc:
        with (
            tc.tile_pool(name="kpool", bufs=2) as kp,
            tc.tile_pool(name="xin0", bufs=2) as xin0,
            tc.tile_pool(name="xin", bufs=3) as xin,
            tc.tile_pool(name="yout", bufs=4) as yp,
            tc.tile_pool(name="ps", bufs=4, space="PSUM") as pp,
        ):
            # K rides the ACT HWDGE ring (idle this early; the SWDGE/Q7 path
            # would add ~2.6us before the first matmul can start).
            k_sb = kp.tile([128, 128], BF16)
            nc.scalar.dma_start(out=k_sb[:], in_=k[:])
            # Warm-up burst: PE runs at half rate until the HAM power
            # throttle sees ~4-5us of sustained activity. A zeroed dummy
            # stationary lets the burst start immediately (no K-DMA wait),
            # so PE is at full rate when the first input tile lands.
            wt = kp.tile([128, 128], BF16)
            nc.vector.memset(wt[:], 0.0)
            ps = pp.tile([128, 1024], F32, tag="ps")
            for w in range(16):
                nc.tensor.matmul(
                    ps[:, :128], wt[:], wt[:], start=True, stop=True
                )
            # One warm-up against K consumes the K-DMA wait off-path.
            nc.tensor.matmul(ps[:, :128], k_sb[:], k_sb[:], start=True, stop=True)
            hh = 0
            for g in range(NGROUPS):
                yt = yp.tile([128, gcols], BF16)
                # Group 0 lands as two 512 KB halves so compute starts ~1.5us
                # earlier; later groups use full 1 MB DMAs (per-DMA overhead
                # on the ring costs ~0.4us each, so fewer is faster).
                if g == 0:
                    xt0 = xin0.tile([128, 2048], BF16)
                    nc.sync.dma_start(out=xt0[:], in_=x[0][:, :2048])
                    xt1 = xin0.tile([128, 2048], BF16)
                    nc.sync.dma_start(out=xt1[:], in_=x[0][:, 2048:])
                    halves = [xt0, xt1]
                else:
                    xt = xin.tile([128, gcols], BF16)
                    nc.sync.dma_start(out=xt[:], in_=x[g])
                    halves = [xt[:, :2048], xt[:, 2048:]]
                # 2-bank PSUM tiles x4 bufs keep PSUM recycling off the
                # critical path; copies alternate DVE / ScalarE (the only
                # PSUM-capable engines) so they drain in parallel.
                for half in range(2):
                    xh = halves[half]
                    for h2 in range(2):
                        ps = pp.tile([128, 1024], F32, tag="ps")
                        for c in range(2):
                            nc.tensor.matmul(
                                ps[:, c * 512 : (c + 1) * 512],
                                k_sb[:],
                                xh[:, h2 * 1024 + c * 512 : h2 * 1024 + (c + 1) * 512],
                                start=True,
                                stop=True,
                            )
                        off = half * 2048 + h2 * 1024
                        if hh % 2 == 0:
                            nc.vector.tensor_copy(yt[:, off : off + 1024], ps[:])
                        else:
                            nc.scalar.copy(yt[:, off : off + 1024], ps[:])
                        hh += 1
                # 1 MB out-DMA per group on the ACT HWDGE ring: by emission
                # order it directly follows this group's last (ACT) copy, so
                # the dispatch never stalls the queue, and HWDGE moves first
                # bytes in ~0.6us vs 2-4.8us on the SWDGE/Q7 path.
                nc.scalar.dma_start(out=y[g], in_=yt[:])
    return _split_multi_waits(nc)


def _get_nc():
    if "nc" not in _NC_CACHE:
        _NC_CACHE["nc"] = _build()
    return _NC_CACHE["nc"]


def _shard(x2d_bf16, i):
    """Core i's slice in device layout x[g, p, cc*512+n] = xs[n, 128c+p]
    with c = 8g+cc (8 superblocks of 512 rows per 1 MB group)."""
    xs = x2d_bf16[i * ROWS_PER_CORE : (i + 1) * ROWS_PER_CORE]  # (512, 4096)
    b = xs.reshape(ROWS_PER_CORE, NSUPER, 128).transpose(2, 1, 0)  # (p, c, n)
    # (p, c, n) -> (g, p, cc, n): c = 8g+cc, 8 superblocks per 1 MB group
    return np.ascontiguousarray(
        b.reshape(128, NGROUPS, NSUPER // NGROUPS, ROWS_PER_CORE).transpose(
            1, 0, 2, 3
        )
    ).reshape(NGROUPS, 128, FREE // NGROUPS)


def _unshard(yb):
    """Invert _shard for one core's output: y[g, p, cc*512+n] = ys[n, 128c+p]
    with c = 8g+cc -> (512, 4096)."""
    yr = yb.reshape(NGROUPS, 128, NSUPER // NGROUPS, ROWS_PER_CORE)
    return np.ascontiguousarray(yr.transpose(3, 0, 2, 1)).reshape(
        ROWS_PER_CORE, H
    )


def _run(x, W_samp, W_init, **run_kwargs):
    x2d = np.asarray(x, dtype=np.float32).reshape(NB, H).astype(NPBF16)
    W1 = np.asarray(W_samp, dtype=np.float32)[:, 0, :]  # (8, 16)
    W2 = np.asarray(W_init, dtype=np.float32)[:, :, 0]  # (16, 8)
    M = W2 @ W1  # (16, 16)
    K = np.ascontiguousarray(
        np.kron(np.eye(SP, dtype=np.float32), M.T)
    ).astype(NPBF16)

    nc = _get_nc()
    in_maps = [{"x": _shard(x2d, i), "k": K} for i in range(N_CORES)]
    res = run_bass_kernel_spmd(nc, in_maps, list(range(N_CORES)), **run_kwargs)
    out = np.concatenate(
        [_unshard(np.asarray(res.results[i]["y"])) for i in range(N_CORES)], axis=0
    ).astype(np.float32)
    return out.reshape(NB, H, 1), res


def kernel(x, W_samp, W_init):
    out, _ = _run(x, W_samp, W_init)
    return out


# revision 30
# speedup vs baseline: 1.0396x; 1.0396x over previous
"""Trainium2 Bass kernel for nn_LSM_IniReconNet.

The reference computes, per contiguous 16-element block of the signal,
z = W1 @ block then y = W2 @ z — i.e. a fixed 16x16 linear map
M = W2 @ W1 applied blockwise. This is pure streaming (memory-bound):
every element is read once, transformed by M, written once.

v2 strategy:
  * bf16 on the wire both directions (rel-err gate is 2e-2; bf16
    end-to-end lands ~4e-3), halving HBM traffic per core to
    4 MB in + 4 MB out.
  * The host lays each core's slice out as [128 partitions = signal
    position within a 128-superblock, free = (superblock, row)] so the
    contraction dim is already on partitions: the device needs NO
    transposes — just DMA in, one bf16 matmul per [128,512] chunk
    against the constant K = kron(I8, M.T), a PSUM->SBUF copy (casting
    back to bf16), and DMA out. The host inverts the permutation.
  * HWDGE DMAs: loads on nc.sync (SP ring), stores on nc.scalar (ACT
    ring) — separate rings, concurrent in/out streaming, ~0.6us fixed
    cost vs ~2us for the SWDGE path.

Sharding: pure data parallel — batch rows split across 8 cores, K
replicated.
"""

import sys

for _p in ("/opt/trn_rl_repo", "/root/.axon_site/_ro/trn_rl_repo"):
    if _p not in sys.path:
        sys.path.insert(0, _p)

import ml_dtypes
import numpy as np

import concourse.bass as bass
import concourse.mybir as mybir
from concourse.bass_utils import run_bass_kernel_spmd
from concourse.tile import TileContext

F32 = mybir.dt.float32
BF16 = mybir.dt.bfloat16
NPBF16 = np.dtype(ml_dtypes.bfloat16)

NB = 4096  # batch
H = 4096  # signal length
BLOCK = 16
SP = 8
N_CORES = 8
ROWS_PER_CORE = NB // N_CORES  # 512
NSUPER = H // 128  # 32 superblocks of 128 positions per row
NGROUPS = 4  # DMA granularity: 1 MB bf16 per group
CHUNKS_PER_GROUP = (NSUPER * ROWS_PER_CORE // 512) // NGROUPS  # 8
FREE = NSUPER * ROWS_PER_CORE  # 16384 free columns on chip

_NC_CACHE = {}


def _split_multi_waits(nc):
    """walrus codegen accepts at most one semaphore wait per instruction
    (beyond what same-queue elision removes). Tile attaches several — most
    notably on the kernel-tail drain. Hoist all but one wait onto wait-only
    NOPs placed immediately before the instruction on the same engine queue.
    """
    ctr = 0
    for fn in nc.m.functions:
        for blk in fn.blocks:
            old = list(blk.instructions)
            if not any(
                i.sync_info is not None and len(i.sync_info.on_wait) > 1 for i in old
            ):
                continue
            new = []
            for inst in old:
                si = inst.sync_info
                if si is not None and len(si.on_wait) > 1:
                    waits = list(si.on_wait)
                    for w in waits[:-1]:
                        ctr += 1
                        new.append(
                            mybir.InstNoOp(
                                name=f"I-waitsplit-{ctr}",
                                sync_info=mybir.SyncInfo(on_wait=[w], on_update=[]),
                                bass_nofuse=True,
                                engine=inst.engine,
                            )
                        )
                    inst.sync_info = mybir.SyncInfo(
                        on_wait=[waits[-1]], on_update=list(si.on_update)
                    )
                new.append(inst)
            blk.instructions = new
    return nc


def _build():
    """Per-core SPMD program.

    x: (128, FREE) bf16 — partition p holds position (128*c + p) of the
    signal for superblock c, free col c*512+n is batch row n.
    k: (128, 128) bf16 = kron(I8, M.T).  y: same layout as x.
    """
    nc = bass.Bass()
    gcols = FREE // NGROUPS  # 4096 free cols per DMA group
    nhalf = FREE // 2048  # 8 output blocks of 512 KB
    # DRAM layouts are block-contiguous so every DMA is a plain
    # contiguous-DRAM <-> [128, N]-SBUF transfer (the cheap 9-desc/engine
    # swizzle; a [128, slice] view of a row-major array would need
    # per-partition descriptors instead).
    x = nc.declare_dram_parameter("x", [NGROUPS, 128, gcols], BF16, isOutput=False)
    k = nc.declare_dram_parameter("k", [128, 128], BF16, isOutput=False)
    y = nc.declare_dram_parameter("y", [NGROUPS, 128, gcols], BF16, isOutput=True)

    with TileContext(nc) as tc:
        with (
            tc.tile_pool(name="kpool", bufs=1) as kp,
            tc.tile_pool(name="xin0", bufs=2) as xin0,
            tc.tile_pool(name="xin", bufs=3) as xin,
            tc.tile_pool(name="yout", bufs=4) as yp,
            tc.tile_pool(name="ps", bufs=4, space="PSUM") as pp,
        ):
            # K rides the ACT HWDGE ring (idle this early; the SWDGE/Q7 path
            # would add ~2.6us before the first matmul can start).
            k_sb = kp.tile([128, 128], BF16)
            nc.scalar.dma_start(out=k_sb[:], in_=k[:])
            # Warm-up burst: PE runs at half rate until the HAM power
            # throttle sees ~4us of sustained activity. Burn the wait for
            # the first input tile on dummy matmuls so real matmuls run at
            # full rate. (The first one also consumes the K-DMA wait.)
            ps = pp.tile([128, 1024], F32, tag="ps")
            for w in range(10):
                nc.tensor.matmul(
                    ps[:, :128], k_sb[:], k_sb[:], start=True, stop=True
                )
            hh = 0
            for g in range(NGROUPS):
                yt = yp.tile([128, gcols], BF16)
                # Group 0 lands as two 512 KB halves so compute starts ~1.5us
                # earlier; later groups use full 1 MB DMAs (per-DMA overhead
                # on the ring costs ~0.4us each, so fewer is faster).
                if g == 0:
                    xt0 = xin0.tile([128, 2048], BF16)
                    nc.sync.dma_start(out=xt0[:], in_=x[0][:, :2048])
                    xt1 = xin0.tile([128, 2048], BF16)
                    nc.sync.dma_start(out=xt1[:], in_=x[0][:, 2048:])
                    halves = [xt0, xt1]
                else:
                    xt = xin.tile([128, gcols], BF16)
                    nc.sync.dma_start(out=xt[:], in_=x[g])
                    halves = [xt[:, :2048], xt[:, 2048:]]
                # 2-bank PSUM tiles x4 bufs keep PSUM recycling off the
                # critical path; copies alternate DVE / ScalarE (the only
                # PSUM-capable engines) so they drain in parallel.
                for half in range(2):
                    xh = halves[half]
                    for h2 in range(2):
                        ps = pp.tile([128, 1024], F32, tag="ps")
                        for c in range(2):
                            nc.tensor.matmul(
                                ps[:, c * 512 : (c + 1) * 512],
                                k_sb[:],
                                xh[:, h2 * 1024 + c * 512 : h2 * 1024 + (c + 1) * 512],
                                start=True,
                                stop=True,
                            )
                        off = half * 2048 + h2 * 1024
                        if hh % 2 == 0:
                            nc.vector.tensor_copy(yt[:, off : off + 1024], ps[:])
                        else:
                            nc.scalar.copy(yt[:, off : off + 1024], ps[:])
                        hh += 1
                # 1 MB out-DMA per group on the ACT HWDGE ring: by emission
                # order it directly follows this group's last (ACT) copy, so
                # the dispatch never stalls the queue, and HWDGE moves first
                # bytes in ~0.6us vs 2-4.8us on the SWDGE/Q7 path.
                nc.scalar.dma_start(out=y[g], in_=yt[:])
    return _split_multi_waits(nc)


def _get_nc():
    if "nc" not in _NC_CACHE:
        _NC_CACHE["nc"] = _build()
    return _NC_CACHE["nc"]


def _shard(x2d_bf16, i):
    """Core i's slice in device layout x[g, p, cc*512+n] = xs[n, 128c+p]
    with c = 8g+cc (8 superblocks of 512 rows per 1 MB group)."""
    xs = x2d_bf16[i * ROWS_PER_CORE : (i + 1) * ROWS_PER_CORE]  # (512, 4096)
    b = xs.reshape(ROWS_PER_CORE, NSUPER, 128).transpose(2, 1, 0)  # (p, c, n)
    # (p, c, n) -> (g, p, cc, n): c = 8g+cc, 8 superblocks per 1 MB group
    return np.ascontiguousarray(
        b.reshape(128, NGROUPS, NSUPER // NGROUPS, ROWS_PER_CORE).transpose(
            1, 0, 2, 3
        )
    ).reshape(NGROUPS, 128, FREE // NGROUPS)


def _unshard(yb):
    """Invert _shard for one core's output: y[g, p, cc*512+n] = ys[n, 128c+p]
    with c = 8g+cc -> (512, 4096)."""
    yr = yb.reshape(NGROUPS, 128, NSUPER // NGROUPS, ROWS_PER_CORE)
    return np.ascontiguousarray(yr.transpose(3, 0, 2, 1)).reshape(
        ROWS_PER_CORE, H
    )


def _run(x, W_samp, W_init, **run_kwargs):
    x2d = np.asarray(x, dtype=np.float32).reshape(NB, H).astype(NPBF16)
    W1 = np.asarray(W_samp, dtype=np.float32)[:, 0, :]  # (8, 16)
    W2 = np.asarray(W_init, dtype=np.float32)[:, :, 0]  # (16, 8)
    M = W2 @ W1  # (16, 16)
    K = np.ascontiguousarray(
        np.kron(np.eye(SP, dtype=np.float32), M.T)
    ).astype(NPBF16)

    nc = _get_nc()
    in_maps = [{"x": _shard(x2d, i), "k": K} for i in range(N_CORES)]
    res = run_bass_kernel_spmd(nc, in_maps, list(range(N_CORES)), **run_kwargs)
    out = np.concatenate(
        [_unshard(np.asarray(res.results[i]["y"])) for i in range(N_CORES)], axis=0
    ).astype(np.float32)
    return out.reshape(NB, H, 1), res


def kernel(x, W_samp, W_init):
    out, _ = _run(x, W_samp, W_init)
    return out


# revision 31
# speedup vs baseline: 1.0666x; 1.0260x over previous
"""Trainium2 Bass kernel for nn_LSM_IniReconNet.

The reference computes, per contiguous 16-element block of the signal,
z = W1 @ block then y = W2 @ z — i.e. a fixed 16x16 linear map
M = W2 @ W1 applied blockwise. This is pure streaming (memory-bound):
every element is read once, transformed by M, written once.

Strategy (measured on HW, ~2.2x over the fp32 baseline):
  * bf16 on the wire both directions (rel-err gate is 2e-2; bf16
    end-to-end lands ~4e-3), halving HBM traffic per core to
    4 MB in + 4 MB out.
  * The host lays each core's slice out as [128 partitions = signal
    position within a 128-superblock, free = (superblock, row)] so the
    contraction dim is already on partitions: the device needs NO
    transposes — just DMA in, one bf16 matmul per [128,512] chunk
    against the constant K = kron(I8, M.T), a PSUM->SBUF copy (casting
    back to bf16, alternating DVE/ScalarE), and DMA out. The host
    inverts the permutation.
  * HWDGE DMAs: loads on nc.sync (SP ring), stores on nc.scalar (ACT
    ring). Measured: the read phase runs ~360 GB/s, the write phase
    ~395 GB/s, but simultaneous read+write drops aggregate to ~317
    GB/s (HBM turnaround), so the schedule intentionally phases input
    mostly before output rather than maximizing overlap.
  * PE warm-up burst against K while the first input tile is in
    flight (HAM power throttle halves matmul rate for the first ~4us;
    zero-operand warm-ups do NOT warm it — it is power-based).

Sharding: pure data parallel — batch rows split across 8 cores, K
replicated.
"""

import sys

for _p in ("/opt/trn_rl_repo", "/root/.axon_site/_ro/trn_rl_repo"):
    if _p not in sys.path:
        sys.path.insert(0, _p)

import ml_dtypes
import numpy as np

import concourse.bass as bass
import concourse.mybir as mybir
from concourse.bass_utils import run_bass_kernel_spmd
from concourse.tile import TileContext

F32 = mybir.dt.float32
BF16 = mybir.dt.bfloat16
NPBF16 = np.dtype(ml_dtypes.bfloat16)

NB = 4096  # batch
H = 4096  # signal length
BLOCK = 16
SP = 8
N_CORES = 8
ROWS_PER_CORE = NB // N_CORES  # 512
NSUPER = H // 128  # 32 superblocks of 128 positions per row
NGROUPS = 4  # DMA granularity: 1 MB bf16 per group
CHUNKS_PER_GROUP = (NSUPER * ROWS_PER_CORE // 512) // NGROUPS  # 8
FREE = NSUPER * ROWS_PER_CORE  # 16384 free columns on chip

_NC_CACHE = {}


def _split_multi_waits(nc):
    """walrus codegen accepts at most one semaphore wait per instruction
    (beyond what same-queue elision removes). Tile attaches several — most
    notably on the kernel-tail drain. Hoist all but one wait onto wait-only
    NOPs placed immediately before the instruction on the same engine queue.
    """
    ctr = 0
    for fn in nc.m.functions:
        for blk in fn.blocks:
            old = list(blk.instructions)
            if not any(
                i.sync_info is not None and len(i.sync_info.on_wait) > 1 for i in old
            ):
                continue
            new = []
            for inst in old:
                si = inst.sync_info
                if si is not None and len(si.on_wait) > 1:
                    waits = list(si.on_wait)
                    for w in waits[:-1]:
                        ctr += 1
                        new.append(
                            mybir.InstNoOp(
                                name=f"I-waitsplit-{ctr}",
                                sync_info=mybir.SyncInfo(on_wait=[w], on_update=[]),
                                bass_nofuse=True,
                                engine=inst.engine,
                            )
                        )
                    inst.sync_info = mybir.SyncInfo(
                        on_wait=[waits[-1]], on_update=list(si.on_update)
                    )
                new.append(inst)
            blk.instructions = new
    return nc


def _build():
    """Per-core SPMD program.

    x: (128, FREE) bf16 — partition p holds position (128*c + p) of the
    signal for superblock c, free col c*512+n is batch row n.
    k: (128, 128) bf16 = kron(I8, M.T).  y: same layout as x.
    """
    nc = bass.Bass()
    gcols = FREE // NGROUPS  # 4096 free cols per DMA group
    nhalf = FREE // 2048  # 8 output blocks of 512 KB
    # DRAM layouts are block-contiguous so every DMA is a plain
    # contiguous-DRAM <-> [128, N]-SBUF transfer (the cheap 9-desc/engine
    # swizzle; a [128, slice] view of a row-major array would need
    # per-partition descriptors instead).
    x = nc.declare_dram_parameter("x", [NGROUPS, 128, gcols], BF16, isOutput=False)
    k = nc.declare_dram_parameter("k", [128, 128], BF16, isOutput=False)
    y = nc.declare_dram_parameter("y", [NGROUPS, 128, gcols], BF16, isOutput=True)

    with TileContext(nc) as tc:
        with (
            tc.tile_pool(name="kpool", bufs=1) as kp,
            tc.tile_pool(name="xin0", bufs=2) as xin0,
            tc.tile_pool(name="xin", bufs=3) as xin,
            tc.tile_pool(name="yout", bufs=4) as yp,
            tc.tile_pool(name="ps", bufs=4, space="PSUM") as pp,
        ):
            # K rides the ACT HWDGE ring (idle this early; the SWDGE/Q7 path
            # would add ~2.6us before the first matmul can start).
            k_sb = kp.tile([128, 128], BF16)
            nc.scalar.dma_start(out=k_sb[:], in_=k[:])
            # Warm-up burst: PE runs at half rate until the HAM power
            # throttle sees ~4us of sustained activity. Burn the wait for
            # the first input tile on dummy matmuls so real matmuls run at
            # full rate. (The first one also consumes the K-DMA wait.)
            ps = pp.tile([128, 1024], F32, tag="ps")
            for w in range(10):
                nc.tensor.matmul(
                    ps[:, :128], k_sb[:], k_sb[:], start=True, stop=True
                )
            hh = 0
            for g in range(NGROUPS):
                yt = yp.tile([128, gcols], BF16)
                # Group 0 lands as two 512 KB halves so compute starts ~1.5us
                # earlier; later groups use full 1 MB DMAs (per-DMA overhead
                # on the ring costs ~0.4us each, so fewer is faster).
                if g == 0:
                    xt0 = xin0.tile([128, 2048], BF16)
                    nc.sync.dma_start(out=xt0[:], in_=x[0][:, :2048])
                    xt1 = xin0.tile([128, 2048], BF16)
                    nc.sync.dma_start(out=xt1[:], in_=x[0][:, 2048:])
                    halves = [xt0, xt1]
                else:
                    xt = xin.tile([128, gcols], BF16)
                    nc.sync.dma_start(out=xt[:], in_=x[g])
                    halves = [xt[:, :2048], xt[:, 2048:]]
                # 2-bank PSUM tiles x4 bufs keep PSUM recycling off the
                # critical path; copies alternate DVE / ScalarE (the only
                # PSUM-capable engines) so they drain in parallel.
                for half in range(2):
                    xh = halves[half]
                    for h2 in range(2):
                        ps = pp.tile([128, 1024], F32, tag="ps")
                        for c in range(2):
                            nc.tensor.matmul(
                                ps[:, c * 512 : (c + 1) * 512],
                                k_sb[:],
                                xh[:, h2 * 1024 + c * 512 : h2 * 1024 + (c + 1) * 512],
                                start=True,
                                stop=True,
                            )
                        off = half * 2048 + h2 * 1024
                        if hh % 2 == 0:
                            nc.vector.tensor_copy(yt[:, off : off + 1024], ps[:])
                        else:
                            nc.scalar.copy(yt[:, off : off + 1024], ps[:])
                        hh += 1
                # 1 MB out-DMA per group on the ACT HWDGE ring: by emission
                # order it directly follows this group's last (ACT) copy, so
                # the dispatch never stalls the queue, and HWDGE moves first
                # bytes in ~0.6us vs 2-4.8us on the SWDGE/Q7 path.
                nc.scalar.dma_start(out=y[g], in_=yt[:])
    return _split_multi_waits(nc)


def _get_nc():
    if "nc" not in _NC_CACHE:
        _NC_CACHE["nc"] = _build()
    return _NC_CACHE["nc"]


def _shard(x2d_bf16, i):
    """Core i's slice in device layout x[g, p, cc*512+n] = xs[n, 128c+p]
    with c = 8g+cc (8 superblocks of 512 rows per 1 MB group)."""
    xs = x2d_bf16[i * ROWS_PER_CORE : (i + 1) * ROWS_PER_CORE]  # (512, 4096)
    b = xs.reshape(ROWS_PER_CORE, NSUPER, 128).transpose(2, 1, 0)  # (p, c, n)
    # (p, c, n) -> (g, p, cc, n): c = 8g+cc, 8 superblocks per 1 MB group
    return np.ascontiguousarray(
        b.reshape(128, NGROUPS, NSUPER // NGROUPS, ROWS_PER_CORE).transpose(
            1, 0, 2, 3
        )
    ).reshape(NGROUPS, 128, FREE // NGROUPS)


def _unshard(yb):
    """Invert _shard for one core's output: y[g, p, cc*512+n] = ys[n, 128c+p]
    with c = 8g+cc -> (512, 4096)."""
    yr = yb.reshape(NGROUPS, 128, NSUPER // NGROUPS, ROWS_PER_CORE)
    return np.ascontiguousarray(yr.transpose(3, 0, 2, 1)).reshape(
        ROWS_PER_CORE, H
    )


def _run(x, W_samp, W_init, **run_kwargs):
    x2d = np.asarray(x, dtype=np.float32).reshape(NB, H).astype(NPBF16)
    W1 = np.asarray(W_samp, dtype=np.float32)[:, 0, :]  # (8, 16)
    W2 = np.asarray(W_init, dtype=np.float32)[:, :, 0]  # (16, 8)
    M = W2 @ W1  # (16, 16)
    K = np.ascontiguousarray(
        np.kron(np.eye(SP, dtype=np.float32), M.T)
    ).astype(NPBF16)

    nc = _get_nc()
    in_maps = [{"x": _shard(x2d, i), "k": K} for i in range(N_CORES)]
    res = run_bass_kernel_spmd(nc, in_maps, list(range(N_CORES)), **run_kwargs)
    out = np.concatenate(
        [_unshard(np.asarray(res.results[i]["y"])) for i in range(N_CORES)], axis=0
    ).astype(np.float32)
    return out.reshape(NB, H, 1), res


def kernel(x, W_samp, W_init):
    out, _ = _run(x, W_samp, W_init)
    return out


# revision 33
# speedup vs baseline: 1.1342x; 1.0633x over previous
"""Trainium2 Bass kernel for nn_LSM_IniReconNet.

The reference computes, per contiguous 16-element block of the signal,
z = W1 @ block then y = W2 @ z — i.e. a fixed 16x16 linear map
M = W2 @ W1 applied blockwise. This is pure streaming (memory-bound):
every element is read once, transformed by M, written once.

Strategy (measured on HW, ~2.2x over the fp32 baseline):
  * bf16 on the wire both directions (rel-err gate is 2e-2; bf16
    end-to-end lands ~4e-3), halving HBM traffic per core to
    4 MB in + 4 MB out.
  * The host lays each core's slice out as [128 partitions = signal
    position within a 128-superblock, free = (superblock, row)] so the
    contraction dim is already on partitions: the device needs NO
    transposes — just DMA in, one bf16 matmul per [128,512] chunk
    against the constant K = kron(I8, M.T), a PSUM->SBUF copy (casting
    back to bf16, alternating DVE/ScalarE), and DMA out. The host
    inverts the permutation.
  * HWDGE DMAs: loads on nc.sync (SP ring), stores on nc.scalar (ACT
    ring). Measured: the read phase runs ~360 GB/s, the write phase
    ~395 GB/s, but simultaneous read+write drops aggregate to ~317
    GB/s (HBM turnaround), so the schedule intentionally phases input
    mostly before output rather than maximizing overlap.
  * PE warm-up burst against K while the first input tile is in
    flight (HAM power throttle halves matmul rate for the first ~4us;
    zero-operand warm-ups do NOT warm it — it is power-based).

Sharding: pure data parallel — batch rows split across 8 cores, K
replicated.
"""

import sys

for _p in ("/opt/trn_rl_repo", "/root/.axon_site/_ro/trn_rl_repo"):
    if _p not in sys.path:
        sys.path.insert(0, _p)

import ml_dtypes
import numpy as np

import concourse.bass as bass
import concourse.mybir as mybir
from concourse.bass_utils import run_bass_kernel_spmd
from concourse.tile import TileContext

F32 = mybir.dt.float32
BF16 = mybir.dt.bfloat16
NPBF16 = np.dtype(ml_dtypes.bfloat16)

NB = 4096  # batch
H = 4096  # signal length
BLOCK = 16
SP = 8
N_CORES = 8
ROWS_PER_CORE = NB // N_CORES  # 512
NSUPER = H // 128  # 32 superblocks of 128 positions per row
NGROUPS = 4  # DMA granularity: 1 MB bf16 per group
CHUNKS_PER_GROUP = (NSUPER * ROWS_PER_CORE // 512) // NGROUPS  # 8
FREE = NSUPER * ROWS_PER_CORE  # 16384 free columns on chip

_NC_CACHE = {}


def _split_multi_waits(nc):
    """walrus codegen accepts at most one semaphore wait per instruction
    (beyond what same-queue elision removes). Tile attaches several — most
    notably on the kernel-tail drain. Hoist all but one wait onto wait-only
    NOPs placed immediately before the instruction on the same engine queue.
    """
    ctr = 0
    for fn in nc.m.functions:
        for blk in fn.blocks:
            old = list(blk.instructions)
            if not any(
                i.sync_info is not None and len(i.sync_info.on_wait) > 1 for i in old
            ):
                continue
            new = []
            for inst in old:
                si = inst.sync_info
                if si is not None and len(si.on_wait) > 1:
                    waits = list(si.on_wait)
                    for w in waits[:-1]:
                        ctr += 1
                        new.append(
                            mybir.InstNoOp(
                                name=f"I-waitsplit-{ctr}",
                                sync_info=mybir.SyncInfo(on_wait=[w], on_update=[]),
                                bass_nofuse=True,
                                engine=inst.engine,
                            )
                        )
                    inst.sync_info = mybir.SyncInfo(
                        on_wait=[waits[-1]], on_update=list(si.on_update)
                    )
                new.append(inst)
            blk.instructions = new
    return nc


def _build():
    """Per-core SPMD program.

    x: (128, FREE) bf16 — partition p holds position (128*c + p) of the
    signal for superblock c, free col c*512+n is batch row n.
    k: (128, 128) bf16 = kron(I8, M.T).  y: same layout as x.
    """
    nc = bass.Bass()
    gcols = FREE // NGROUPS  # 4096 free cols per DMA group
    nhalf = FREE // 2048  # 8 output blocks of 512 KB
    # DRAM layouts are block-contiguous so every DMA is a plain
    # contiguous-DRAM <-> [128, N]-SBUF transfer (the cheap 9-desc/engine
    # swizzle; a [128, slice] view of a row-major array would need
    # per-partition descriptors instead).
    x = nc.declare_dram_parameter("x", [NGROUPS, 128, gcols], BF16, isOutput=False)
    k = nc.declare_dram_parameter("k", [128, 128], BF16, isOutput=False)
    y = nc.declare_dram_parameter("y", [NGROUPS, 128, gcols], BF16, isOutput=True)

    with TileContext(nc) as tc:
        with (
            tc.tile_pool(name="kpool", bufs=1) as kp,
            tc.tile_pool(name="warm", bufs=1) as wp,
            tc.tile_pool(name="xin0", bufs=2) as xin0,
            tc.tile_pool(name="xin", bufs=3) as xin,
            tc.tile_pool(name="yout", bufs=4) as yp,
            tc.tile_pool(name="ps", bufs=4, space="PSUM") as pp,
        ):
            # K rides the ACT HWDGE ring (idle this early; the SWDGE/Q7 path
            # would add ~2.6us before the first matmul can start).
            k_sb = kp.tile([128, 128], BF16)
            nc.scalar.dma_start(out=k_sb[:], in_=k[:])
            # Warm-up burst: PE runs at half rate until the HAM power
            # throttle sees sustained *power* (not mere activity — zero or
            # narrow operands don't count). Three full-width 512-col
            # matmuls of K against a nonzero constant fill the K-DMA wait
            # and exercise the whole array. (First one consumes the K wait.)
            wm = wp.tile([128, 512], BF16)
            nc.vector.memset(wm[:], 1.375)
            ps = pp.tile([128, 1024], F32, tag="ps")
            for w in range(3):
                nc.tensor.matmul(
                    ps[:, :512], k_sb[:], wm[:], start=True, stop=True
                )
            hh = 0
            for g in range(NGROUPS):
                yt = yp.tile([128, gcols], BF16)
                # Group 0 lands as two 512 KB halves so compute starts ~1.5us
                # earlier; later groups use full 1 MB DMAs (per-DMA overhead
                # on the ring costs ~0.4us each, so fewer is faster).
                if g == 0:
                    xt0 = xin0.tile([128, 2048], BF16)
                    nc.sync.dma_start(out=xt0[:], in_=x[0][:, :2048])
                    xt1 = xin0.tile([128, 2048], BF16)
                    nc.sync.dma_start(out=xt1[:], in_=x[0][:, 2048:])
                    halves = [xt0, xt1]
                else:
                    xt = xin.tile([128, gcols], BF16)
                    nc.sync.dma_start(out=xt[:], in_=x[g])
                    halves = [xt[:, :2048], xt[:, 2048:]]
                # 2-bank PSUM tiles x4 bufs keep PSUM recycling off the
                # critical path; copies alternate DVE / ScalarE (the only
                # PSUM-capable engines) so they drain in parallel.
                for half in range(2):
                    xh = halves[half]
                    for h2 in range(2):
                        ps = pp.tile([128, 1024], F32, tag="ps")
                        for c in range(2):
                            nc.tensor.matmul(
                                ps[:, c * 512 : (c + 1) * 512],
                                k_sb[:],
                                xh[:, h2 * 1024 + c * 512 : h2 * 1024 + (c + 1) * 512],
                                start=True,
                                stop=True,
                            )
                        off = half * 2048 + h2 * 1024
                        if hh % 2 == 0:
                            nc.vector.tensor_copy(yt[:, off : off + 1024], ps[:])
                        else:
                            nc.scalar.copy(yt[:, off : off + 1024], ps[:])
                        hh += 1
                # 1 MB out-DMA per group on the ACT HWDGE ring: by emission
                # order it directly follows this group's last (ACT) copy, so
                # the dispatch never stalls the queue, and HWDGE moves first
                # bytes in ~0.6us vs 2-4.8us on the SWDGE/Q7 path.
                nc.scalar.dma_start(out=y[g], in_=yt[:])
    return _split_multi_waits(nc)


def _get_nc():
    if "nc" not in _NC_CACHE:
        _NC_CACHE["nc"] = _build()
    return _NC_CACHE["nc"]


def _shard(x2d_bf16, i):
    """Core i's slice in device layout x[g, p, cc*512+n] = xs[n, 128c+p]
    with c = 8g+cc (8 superblocks of 512 rows per 1 MB group)."""
    xs = x2d_bf16[i * ROWS_PER_CORE : (i + 1) * ROWS_PER_CORE]  # (512, 4096)
    b = xs.reshape(ROWS_PER_CORE, NSUPER, 128).transpose(2, 1, 0)  # (p, c, n)
    # (p, c, n) -> (g, p, cc, n): c = 8g+cc, 8 superblocks per 1 MB group
    return np.ascontiguousarray(
        b.reshape(128, NGROUPS, NSUPER // NGROUPS, ROWS_PER_CORE).transpose(
            1, 0, 2, 3
        )
    ).reshape(NGROUPS, 128, FREE // NGROUPS)


def _unshard(yb):
    """Invert _shard for one core's output: y[g, p, cc*512+n] = ys[n, 128c+p]
    with c = 8g+cc -> (512, 4096)."""
    yr = yb.reshape(NGROUPS, 128, NSUPER // NGROUPS, ROWS_PER_CORE)
    return np.ascontiguousarray(yr.transpose(3, 0, 2, 1)).reshape(
        ROWS_PER_CORE, H
    )


def _run(x, W_samp, W_init, **run_kwargs):
    x2d = np.asarray(x, dtype=np.float32).reshape(NB, H).astype(NPBF16)
    W1 = np.asarray(W_samp, dtype=np.float32)[:, 0, :]  # (8, 16)
    W2 = np.asarray(W_init, dtype=np.float32)[:, :, 0]  # (16, 8)
    M = W2 @ W1  # (16, 16)
    K = np.ascontiguousarray(
        np.kron(np.eye(SP, dtype=np.float32), M.T)
    ).astype(NPBF16)

    nc = _get_nc()
    in_maps = [{"x": _shard(x2d, i), "k": K} for i in range(N_CORES)]
    res = run_bass_kernel_spmd(nc, in_maps, list(range(N_CORES)), **run_kwargs)
    out = np.concatenate(
        [_unshard(np.asarray(res.results[i]["y"])) for i in range(N_CORES)], axis=0
    ).astype(np.float32)
    return out.reshape(NB, H, 1), res


def kernel(x, W_samp, W_init):
    out, _ = _run(x, W_samp, W_init)
    return out
